# revision 1
# baseline (speedup 1.0000x reference)
"""Paged-attention decode kernel for 8 TRN2 NeuronCores (Bass/Tile).

Problem: nn_Attention_15229954031958 (sparse_attention, memory-bound).
  q [32, 32, 128] f32, k/v_cache [8192, 16, 8, 128] f32,
  block_tables [32, 256] i32, context_lens [32] i32 -> out [32, 32, 128] f32.

Sharding: tensor-parallel over KV heads. Core c holds the head-c slice of
both caches plus q heads 4c..4c+3; no cross-core communication. Every
core runs the same graph (trip counts baked from context_lens, identical
on all cores), so SPMD is trivially satisfied.

Layout ("ilv" scheme). K is stored fp8-e3m4, one 2KB row per page, host
pre-swizzled as (t, d) -> (t//2, d, t%2). The transposed dma_gather moves
16-bit units through the xbar, so the gathered K tile lands as
[d=128, j=8, page, c=2] with token t = 2j + c; a flat 128-column slice
over (page, c) is CONTIGUOUS, giving the QK matmul an FWL-eligible
stationary operand. Score rows are therefore (page, parity)-interleaved
pairs over 64-page half-groups. V is stored bf16 as split half-pages
(row 2i = even tokens of page i, row 2i+1 = odd tokens); its natural
gather with interleaved indices produces V rows in exactly the same
(page, parity) order, so the PV contraction lines up row-for-row.

Per sequence (nb = ceil(ctx/16) pages, nhg = ceil2(nb)/64 half-groups):
  - per half-group hg and slot-pair j: scores[128, 4] =
      ktile[:, j, 128hg:128hg+128]^T @ qT[:, 4b:4b+4]   (PSUM [128, 32])
  - one batched exp per half-group on ScalarE -> bf16 p
  - PV: out[128, 4] += vtile[0:jt2, hg, 128j:+128]^T @ p[0:jt2, 4j:4j+4]
    with jt2 the static count of leading valid (page, parity) rows;
    V stationary (128-col weight -> FWL), p streams 4 cols.
  - den: p^T @ ones -> [32, 1] PSUM accumulated over half-groups,
    collapsed to [4, 1] with a selector matmul; epilogue transposes the
    PV accumulator on the PE and scales by 1/den.
"""

import os
import numpy as np
import ml_dtypes

BLOCK = 16
D = 128
B = 32
H = 32
KVH = 8
G = 4
NBLK = 8192
MAXB = 256
SCALE = 0.08838834764831845
BF16 = ml_dtypes.bfloat16
F8E3 = ml_dtypes.float8_e3m4

_GRAPH_CACHE = {}


def _round_up(x, m):
    return (x + m - 1) // m * m


def _jt2(ctx, hg, j):
    """Leading valid (page, parity) rows of half-group hg, slot-pair j."""
    fb, rem = ctx // BLOCK, ctx % BLOCK
    n0 = min(max(fb + (1 if 2 * j < rem else 0) - 64 * hg, 0), 64)
    n1 = min(max(fb + (1 if 2 * j + 1 < rem else 0) - 64 * hg, 0), 64)
    return n0 + n1


def _build_graph(ctx_lens, repeat=1, bufs=(3, 3, 3, 2, 2, 2), mode="full",
                 vq=1, vdt="bf16", sp=1):
    """Build + compile the SPMD graph for the given context lengths.

    repeat > 1 duplicates the whole body (for timing: slope difference
    between repeat=R and repeat=1 isolates pure HW time).
    mode: "full" | "dma"/"dmak"/"dmav" (gathers only) | "compute".
    vq: SWDGE queue for the V gather (K is always queue 0).
    vdt: "bf16" | "fp8" (e3m4) for the V cache."""
    import concourse.bass as bass
    import concourse.tile as tile
    from concourse import bacc, mybir
    from concourse.masks import make_identity
    from contextlib import ExitStack

    kb, vb, pb, eb, sb, ob = bufs
    nbs = [max(1, -(-int(c) // BLOCK)) for c in ctx_lens]
    nks = [_round_up(nb, 128) for nb in nbs]
    # idx columns per seq: nk/16 for K, 2nk/16 for V
    idx_cols = sum(3 * nk // 16 for nk in nks)
    # process sequences largest-first: best prefetch ramp at the start and a
    # minimal non-overlapped tail (last gather is the smallest sequence)
    order = list(np.argsort(-np.asarray(nbs), kind="stable"))

    nc = bacc.Bacc("TRN2", target_bir_lowering=False, debug=False,
                   num_swdge_queues=(2 if vq else 1))

    vdtype = mybir.dt.float8e3 if vdt == "fp8" else mybir.dt.bfloat16
    k_src = nc.dram_tensor("k_src", [NBLK, BLOCK * D], mybir.dt.float8e3,
                           kind="ExternalInput").ap()
    v_src = nc.dram_tensor("v_src", [2 * NBLK, 8 * D], vdtype,
                           kind="ExternalInput").ap()
    qT_d = nc.dram_tensor("qT", [D, B * G], mybir.dt.float32,
                          kind="ExternalInput").ap()
    idx_d = nc.dram_tensor("idx", [128, idx_cols], mybir.dt.int16,
                           kind="ExternalInput").ap()
    sel_d = nc.dram_tensor("sel", [8 * G, G], mybir.dt.float32,
                           kind="ExternalInput").ap()
    mask_d = nc.dram_tensor("mask", [B, 128, 8 * G], mybir.dt.int8,
                            kind="ExternalInput").ap()
    out_d = nc.dram_tensor("out", [B, G, D], mybir.dt.float32,
                           kind="ExternalOutput").ap()

    with tile.TileContext(nc) as tc, ExitStack() as ctx:
        const = ctx.enter_context(tc.tile_pool(name="const", bufs=1))
        kpool = ctx.enter_context(tc.tile_pool(name="kpool", bufs=kb))
        vpool = ctx.enter_context(tc.tile_pool(name="vpool", bufs=vb))
        ppool = ctx.enter_context(tc.tile_pool(name="ppool", bufs=pb))
        epool = ctx.enter_context(tc.tile_pool(name="epool", bufs=eb))
        spsum = ctx.enter_context(tc.tile_pool(name="spsum", bufs=sb, space="PSUM"))
        opsum = ctx.enter_context(tc.tile_pool(name="opsum", bufs=ob, space="PSUM"))
        dpsum = ctx.enter_context(tc.tile_pool(name="dpsum", bufs=1, space="PSUM"))
        tpsum = ctx.enter_context(tc.tile_pool(name="tpsum", bufs=1, space="PSUM"))
        npsum = ctx.enter_context(tc.tile_pool(name="npsum", bufs=1, space="PSUM"))

        qT_sb = const.tile([128, B * G], mybir.dt.float32)
        nc.sync.dma_start(out=qT_sb[:], in_=qT_d[:])
        qT_b = const.tile([128, B * G], mybir.dt.bfloat16)
        nc.vector.tensor_copy(out=qT_b[:], in_=qT_sb[:])

        ident = const.tile([128, 128], mybir.dt.float32)
        make_identity(nc, ident[:])

        ones_b = const.tile([128, 1], mybir.dt.bfloat16)
        nc.vector.memset(ones_b[:], 1.0)

        sel_sb = const.tile([8 * G, G], mybir.dt.float32)
        nc.sync.dma_start(out=sel_sb[:], in_=sel_d[:])

        idx_sb = const.tile([128, idx_cols], mybir.dt.int16)
        nc.sync.dma_start(out=idx_sb[:], in_=idx_d[:])

        for _rep in range(repeat):
          col = 0
          for bi, b in enumerate(order):
            ctx_b = int(ctx_lens[b])
            nb, nk = nbs[b], nks[b]
            nhg = nk // 64
            rem = ctx_b % BLOCK

            ktile = kpool.tile([128, 16 * nk], mybir.dt.float8e3, tag="kt")
            # gather AP (shape-check only); physical layout is [d, j, page, c]
            kap_g = ktile[:].rearrange("p (t n) -> p t n", t=16)
            # contiguous view for the QK stationary: [d, j, (page c)]
            kap = ktile[:].rearrange("p (j m) -> p j m", j=8)
            vtile = vpool.tile([128, nhg * 8 * D], vdtype, tag="vt")
            vap = vtile[:].rearrange("p (g n) -> p g n", g=nhg)
            if mode != "compute":
                if mode != "dmav":
                    nc.gpsimd.dma_gather(
                        out_ap=kap_g,
                        in_ap=k_src[:],
                        idxs_ap=idx_sb[:, col:col + nk // 16],
                        num_idxs=nk,
                        num_idxs_reg=nb,
                        elem_size=BLOCK * D,
                        transpose=True,
                        single_packet=bool(sp),
                    )
                if mode != "dmak":
                    nc.gpsimd.dma_gather(
                        out_ap=vap,
                        in_ap=v_src[:],
                        idxs_ap=idx_sb[:, col + nk // 16:col + 3 * nk // 16],
                        num_idxs=2 * nk,
                        num_idxs_reg=2 * nb,
                        elem_size=8 * D,
                        transpose=False,
                        queue_num=vq,
                        single_packet=bool(sp),
                    )
            else:
                # compute mode: every logical tile needs a write for Tile
                # validation; a sliver is enough (timing only, not numerics)
                nc.vector.memset(ktile[:, 0:16], 0.0)
                nc.vector.memset(vtile[:, 0:16], 0.0)
            col += 3 * nk // 16
            if mode.startswith("dma"):
                # minimal consumer so the gathers aren't dead: copy a sliver
                sliver = epool.tile([128, 4], mybir.dt.float32, tag="slv")
                if mode != "dmav":
                    nc.vector.tensor_copy(out=sliver[:], in_=kap[:, 0, 0:4])
                if mode != "dmak":
                    nc.vector.tensor_copy(out=sliver[:], in_=vap[:, 0, 0:4])
                if bi == B - 1:
                    fin0 = epool.tile([G, D], mybir.dt.float32, tag="fin")
                    nc.vector.memset(fin0[:], 0.0)
                    for bb in range(B):
                        nc.sync.dma_start(out=out_d[bb], in_=fin0[:])
                continue

            out_ps = opsum.tile([128, G], mybir.dt.float32, tag="ops")
            den_ps = dpsum.tile([8 * G, 1], mybir.dt.float32, tag="dps")

            hgs = [hg for hg in range(nhg)
                   if any(_jt2(ctx_b, hg, j) > 0 for j in range(8))]
            last_hg = hgs[-1]
            n_pv = sum(1 for hg in hgs for j in range(8)
                       if _jt2(ctx_b, hg, j) > 0)
            pv_i = 0
            first_pv = True
            for hgi, hg in enumerate(hgs):
                scores_ps = spsum.tile([128, 8 * G], mybir.dt.float32, tag="sps")
                for j in range(8):
                    if _jt2(ctx_b, hg, j) == 0:
                        continue
                    nc.tensor.matmul(
                        scores_ps[:, 4 * j:4 * j + 4],
                        lhsT=kap[:, j, 128 * hg:128 * hg + 128],
                        rhs=qT_b[:, 4 * b:4 * b + 4],
                        start=True, stop=True,
                    )
                # is any (row, col) of this half-group's p tile garbage?
                partial = (hg == last_hg) and (2 * nb - 128 * hg < 128
                                               or rem > 0)
                ptile = ppool.tile([128, 8 * G], mybir.dt.bfloat16, tag="pt")
                if not partial:
                    nc.scalar.activation(ptile[:], scores_ps[:],
                                         mybir.ActivationFunctionType.Exp)
                else:
                    # exp into a temp, then keep only in-context entries so
                    # garbage (possibly NaN/inf) never reaches den/PV.
                    ptmp = ppool.tile([128, 8 * G], mybir.dt.bfloat16, tag="ptmp")
                    nc.scalar.activation(ptmp[:], scores_ps[:],
                                         mybir.ActivationFunctionType.Exp)
                    msk = epool.tile([128, 8 * G], mybir.dt.int8, tag="msk")
                    nc.sync.dma_start(out=msk[:], in_=mask_d[b])
                    nc.vector.memset(ptile[:], 0.0)
                    nc.vector.copy_predicated(ptile[:], msk[:], ptmp[:])
                # denominator contribution of this half-group
                nc.tensor.matmul(
                    den_ps[:],
                    lhsT=ptile[:],
                    rhs=ones_b[:],
                    start=(hgi == 0), stop=(hg == last_hg),
                )
                # PV accumulation: V stationary (128-col weight -> FWL), p
                # streams as the 4-col moving operand.
                for j in range(8):
                    jt = _jt2(ctx_b, hg, j)
                    if jt == 0:
                        continue
                    pv_i += 1
                    nc.tensor.matmul(
                        out_ps[:],
                        lhsT=vap[0:jt, hg, D * j:D * j + D],
                        rhs=ptile[0:jt, 4 * j:4 * j + 4],
                        start=first_pv, stop=(pv_i == n_pv),
                    )
                    first_pv = False

            # epilogue: out_ps [128,4] -> transpose -> scale by 1/den -> stage
            o_sb = epool.tile([128, G], mybir.dt.float32, tag="osb")
            nc.vector.tensor_copy(out=o_sb[:], in_=out_ps[:])
            oT_ps = tpsum.tile([G, 128], mybir.dt.float32, tag="otp")
            nc.tensor.transpose(oT_ps[:], o_sb[:], ident[:])

            den_sb = epool.tile([8 * G, 1], mybir.dt.float32, tag="dsb")
            nc.vector.tensor_copy(out=den_sb[:], in_=den_ps[:])
            den4_ps = npsum.tile([G, 1], mybir.dt.float32, tag="d4p")
            nc.tensor.matmul(den4_ps[:], lhsT=sel_sb[:], rhs=den_sb[:],
                             start=True, stop=True)
            den4_sb = epool.tile([G, 1], mybir.dt.float32, tag="d4s")
            nc.vector.tensor_copy(out=den4_sb[:], in_=den4_ps[:])
            rcp = epool.tile([G, 1], mybir.dt.float32, tag="rcp")
            nc.vector.reciprocal(rcp[:], den4_sb[:])

            fin = epool.tile([G, D], mybir.dt.float32, tag="fin")
            nc.vector.tensor_tensor(
                out=fin[:],
                in0=oT_ps[:],
                in1=rcp[:].to_broadcast([G, D]),
                op=mybir.AluOpType.mult,
            )
            nc.sync.dma_start(out=out_d[b], in_=fin[:])

    nc.compile()
    return nc


def _prep_host(q, k_cache, v_cache, block_tables, context_lens, vdt="bf16"):
    """Shard + reformat inputs for the 8 cores. Returns in_maps list."""
    VDT = F8E3 if vdt == "fp8" else BF16
    ctx_lens = np.asarray(context_lens, dtype=np.int64)
    bt = np.asarray(block_tables, dtype=np.int64)
    nbs = [max(1, -(-int(c) // BLOCK)) for c in ctx_lens]
    nks = [_round_up(nb, 128) for nb in nbs]
    idx_cols = sum(3 * nk // 16 for nk in nks)

    # idx columns are packed in the same largest-first order the graph
    # builder iterates sequences in (see _build_graph): K block then V block.
    order = list(np.argsort(-np.asarray(nbs), kind="stable"))
    idx16 = np.full((16, idx_cols), -1, dtype=np.int16)
    col = 0
    for b in order:
        nb, nk = nbs[b], nks[b]
        ids = np.full(nk, -1, dtype=np.int16)
        ids[:nb] = bt[b, :nb].astype(np.int16)
        idx16[:, col:col + nk // 16] = ids.reshape(nk // 16, 16).T
        col += nk // 16
        iv = np.full(2 * nk, -1, dtype=np.int16)
        iv[0:2 * nb:2] = (2 * bt[b, :nb]).astype(np.int16)
        iv[1:2 * nb:2] = (2 * bt[b, :nb] + 1).astype(np.int16)
        idx16[:, col:col + 2 * nk // 16] = iv.reshape(2 * nk // 16, 16).T
        col += 2 * nk // 16
    idx_all = np.tile(idx16, (8, 1))  # replicate across the 8 Q7 cores

    sel = np.zeros((8 * G, G), dtype=np.float32)
    for i in range(8 * G):
        sel[i, i % G] = 1.0

    # validity mask of the LAST half-group of each sequence:
    # mask[b, r, 4j+g] = 1 iff token (page 64*hg + r//2, slot 2j + r%2) < ctx
    mask = np.zeros((B, 128, 8 * G), dtype=np.int8)
    rv = np.arange(128)
    jv = np.arange(8)
    for b in range(B):
        ctx_b = int(ctx_lens[b])
        hg = (nbs[b] - 1) // 64
        pos = (BLOCK * (64 * hg + rv[:, None] // 2)
               + 2 * jv[None, :] + (rv[:, None] % 2))  # [128, 8]
        m = (pos < ctx_b).astype(np.int8)
        mask[b] = np.repeat(m, G, axis=1)

    q = np.asarray(q, dtype=np.float32)
    kc = np.asarray(k_cache, dtype=np.float32)
    vc = np.asarray(v_cache, dtype=np.float32)

    in_maps = []
    for c in range(KVH):
        ks = np.ascontiguousarray(kc[:, :, c, :])  # [NBLK, 16, 128] f32
        # page layout (t, d) -> (t//2, d, t%2) so the 16-bit-granularity
        # transposed gather lands K^T as [d, j, page, t%2]
        k_shard = np.ascontiguousarray(
            ks.reshape(NBLK, 8, 2, 128).transpose(0, 1, 3, 2)
        ).astype(F8E3).reshape(NBLK, BLOCK * D)
        vs = np.ascontiguousarray(vc[:, :, c, :]).astype(VDT)  # [NBLK,16,128]
        v_shard = np.empty((2 * NBLK, 8 * D), dtype=VDT)
        v_shard[0::2] = vs[:, 0::2, :].reshape(NBLK, 8 * D)
        v_shard[1::2] = vs[:, 1::2, :].reshape(NBLK, 8 * D)
        qs = np.ascontiguousarray(q[:, G * c:G * c + G, :] * SCALE)  # [32,4,128]
        qT = np.ascontiguousarray(qs.reshape(B * G, D).T.astype(np.float32))
        in_maps.append({
            "k_src": k_shard,
            "v_src": v_shard,
            "qT": qT,
            "idx": idx_all,
            "sel": sel,
            "mask": mask,
        })
    return in_maps


def _get_graph(context_lens, repeat=1, bufs=(3, 3, 3, 2, 2, 2), mode="full",
               vq=1, vdt="bf16", sp=1):
    key = (bytes(np.asarray(context_lens, dtype=np.int32)), repeat, bufs, mode,
           vq, vdt, sp)
    if key not in _GRAPH_CACHE:
        _GRAPH_CACHE[key] = _build_graph(
            np.asarray(context_lens, dtype=np.int64), repeat=repeat, bufs=bufs,
            mode=mode, vq=vq, vdt=vdt, sp=sp)
    return _GRAPH_CACHE[key]


def kernel_run(q, k_cache, v_cache, block_tables, context_lens, trace=False):
    """Run on the 8 NeuronCores; returns (out, BassKernelResults)."""
    import time
    from concourse.bass_utils import run_bass_kernel_spmd

    nc = _get_graph(context_lens)
    in_maps = _prep_host(q, k_cache, v_cache, block_tables, context_lens)
    last_exc = None
    for attempt in range(3):
        try:
            res = run_bass_kernel_spmd(nc, in_maps, core_ids=list(range(8)),
                                       trace=trace)
            break
        except Exception as e:  # transient device wedge (e.g. NRT_EXEC_UNIT_
            last_exc = e        # UNRECOVERABLE) — back off and retry
            time.sleep(5 * (attempt + 1))
    else:
        raise last_exc
    outs = [np.asarray(r["out"], dtype=np.float32) for r in res.results]
    out = np.concatenate(outs, axis=1).reshape(B, H, D)
    return out, res


def kernel(q, k_cache, v_cache, block_tables, context_lens):
    out, _ = kernel_run(q, k_cache, v_cache, block_tables, context_lens,
                        trace=False)
    return out



# revision 17
# speedup vs baseline: 1.3763x; 1.3763x over previous
"""Paged-attention decode kernel for 8 TRN2 NeuronCores (Bass/Tile).

Problem: nn_Attention_15229954031958 (sparse_attention, memory-bound).
  q [32, 32, 128] f32, k/v_cache [8192, 16, 8, 128] f32,
  block_tables [32, 256] i32, context_lens [32] i32 -> out [32, 32, 128] f32.

Sharding: tensor-parallel over KV heads. Core c holds the head-c slice of
both caches plus q heads 4c..4c+3; no cross-core communication. Every
core runs the same graph (trip counts baked from context_lens, identical
on all cores), so SPMD is trivially satisfied.

Layout ("ilv" scheme). K is stored fp8-e3m4, one 2KB row per page, host
pre-swizzled as (t, d) -> (t//2, d, t%2). The transposed dma_gather moves
16-bit units through the xbar, so the gathered K tile lands as
[d=128, j=8, page, c=2] with token t = 2j + c; a flat 128-column slice
over (page, c) is CONTIGUOUS, giving the QK matmul an FWL-eligible
stationary operand. Score rows are therefore (page, parity)-interleaved
pairs over 64-page half-groups. V is stored bf16 as split half-pages
(row 2i = even tokens of page i, row 2i+1 = odd tokens); its natural
gather with interleaved indices produces V rows in exactly the same
(page, parity) order, so the PV contraction lines up row-for-row.

Per sequence (nb = ceil(ctx/16) pages, nhg = ceil2(nb)/64 half-groups):
  - per half-group hg and slot-pair j: scores[128, 4] =
      ktile[:, j, 128hg:128hg+128]^T @ qT[:, 4b:4b+4]   (PSUM [128, 32])
  - one batched exp per half-group on ScalarE -> bf16 p
  - PV: out[128, 4] += vtile[0:jt2, hg, 128j:+128]^T @ p[0:jt2, 4j:4j+4]
    with jt2 the static count of leading valid (page, parity) rows;
    V stationary (128-col weight -> FWL), p streams 4 cols.
  - den: p^T @ ones -> [32, 1] PSUM accumulated over half-groups,
    collapsed to [4, 1] with a selector matmul; epilogue transposes the
    PV accumulator on the PE and scales by 1/den.
"""

import os
import numpy as np
import ml_dtypes

BLOCK = 16
D = 128
B = 32
H = 32
KVH = 8
G = 4
NBLK = 8192
MAXB = 256
SCALE = 0.08838834764831845
BF16 = ml_dtypes.bfloat16
F8E3 = ml_dtypes.float8_e3m4

_GRAPH_CACHE = {}


def _round_up(x, m):
    return (x + m - 1) // m * m


def _jt2(ctx, hg, j):
    """Leading valid (page, parity) rows of half-group hg, slot-pair j."""
    fb, rem = ctx // BLOCK, ctx % BLOCK
    n0 = min(max(fb + (1 if 2 * j < rem else 0) - 64 * hg, 0), 64)
    n1 = min(max(fb + (1 if 2 * j + 1 < rem else 0) - 64 * hg, 0), 64)
    return n0 + n1


def _build_graph(ctx_lens, repeat=1, bufs=(3, 3, 3, 2, 2, 2), mode="full",
                 vq=1, vdt="bf16", sp=1, qmap=""):
    """Build + compile the SPMD graph for the given context lengths.

    repeat > 1 duplicates the whole body (for timing: slope difference
    between repeat=R and repeat=1 isolates pure HW time).
    mode: "full" | "dma"/"dmak"/"dmav" (gathers only) | "compute".
    vq: SWDGE queue for the V gather (K is always queue 0).
    vdt: "bf16" | "fp8" (e3m4) for the V cache.
    qmap: "kAvB" -> K gathers round-robin on queues [0, A), V gathers on
    queues [A, A+B). Overrides vq. Empty = legacy (K on 0, V on vq)."""
    import concourse.bass as bass
    import concourse.tile as tile
    from concourse import bacc, mybir
    from concourse.masks import make_identity
    from contextlib import ExitStack

    kb, vb, pb, eb, sb, ob = bufs
    nbs = [max(1, -(-int(c) // BLOCK)) for c in ctx_lens]
    nks = [_round_up(nb, 128) for nb in nbs]
    # V idx stream is exact-size (2nb rounded to 16), not nk-padded
    nvs = [_round_up(2 * nb, 16) for nb in nbs]
    # idx columns per seq: nk/16 for K, nv/16 for V
    idx_cols = sum(nk // 16 + nv // 16 for nk, nv in zip(nks, nvs))
    # process sequences largest-first: best prefetch ramp at the start and a
    # minimal non-overlapped tail (last gather is the smallest sequence)
    order = list(np.argsort(-np.asarray(nbs), kind="stable"))

    if qmap.startswith("m"):
        nkq, nvq = 0, 0
        n_queues = int(qmap[1])
    elif qmap:
        nkq, nvq = int(qmap[1]), int(qmap[3])
        n_queues = nkq + nvq
    else:
        nkq, nvq = 0, 0
        n_queues = 2 if vq else 1
    nc = bacc.Bacc("TRN2", target_bir_lowering=False, debug=False,
                   num_swdge_queues=n_queues)

    vdtype = mybir.dt.float8e3 if vdt == "fp8" else mybir.dt.bfloat16
    k_src = nc.dram_tensor("k_src", [NBLK, BLOCK * D], mybir.dt.float8e3,
                           kind="ExternalInput").ap()
    v_src = nc.dram_tensor("v_src", [2 * NBLK, 8 * D], vdtype,
                           kind="ExternalInput").ap()
    qT_d = nc.dram_tensor("qT", [D, B * G], mybir.dt.float32,
                          kind="ExternalInput").ap()
    idx_d = nc.dram_tensor("idx", [128, idx_cols], mybir.dt.int16,
                           kind="ExternalInput").ap()
    sel_d = nc.dram_tensor("sel", [8 * G, G], mybir.dt.float32,
                           kind="ExternalInput").ap()
    mask_d = nc.dram_tensor("mask", [B, 128, 8 * G], mybir.dt.int8,
                            kind="ExternalInput").ap()
    out_d = nc.dram_tensor("out", [B, G, D], mybir.dt.float32,
                           kind="ExternalOutput").ap()

    with tile.TileContext(nc) as tc, ExitStack() as ctx:
        const = ctx.enter_context(tc.tile_pool(name="const", bufs=1))
        kpool = ctx.enter_context(tc.tile_pool(name="kpool", bufs=kb))
        vpool = ctx.enter_context(tc.tile_pool(name="vpool", bufs=vb))
        ppool = ctx.enter_context(tc.tile_pool(name="ppool", bufs=pb))
        epool = ctx.enter_context(tc.tile_pool(name="epool", bufs=eb))
        spsum = ctx.enter_context(tc.tile_pool(name="spsum", bufs=sb, space="PSUM"))
        opsum = ctx.enter_context(tc.tile_pool(name="opsum", bufs=ob, space="PSUM"))
        dpsum = ctx.enter_context(tc.tile_pool(name="dpsum", bufs=1, space="PSUM"))
        tpsum = ctx.enter_context(tc.tile_pool(name="tpsum", bufs=1, space="PSUM"))
        npsum = ctx.enter_context(tc.tile_pool(name="npsum", bufs=1, space="PSUM"))

        qT_sb = const.tile([128, B * G], mybir.dt.float32)
        nc.sync.dma_start(out=qT_sb[:], in_=qT_d[:])
        qT_b = const.tile([128, B * G], mybir.dt.bfloat16)
        nc.vector.tensor_copy(out=qT_b[:], in_=qT_sb[:])

        ident = const.tile([128, 128], mybir.dt.float32)
        make_identity(nc, ident[:])

        ones_b = const.tile([128, 1], mybir.dt.bfloat16)
        nc.vector.memset(ones_b[:], 1.0)

        sel_sb = const.tile([8 * G, G], mybir.dt.float32)
        nc.sync.dma_start(out=sel_sb[:], in_=sel_d[:])

        idx_sb = const.tile([128, idx_cols], mybir.dt.int16)
        nc.sync.dma_start(out=idx_sb[:], in_=idx_d[:])

        for _rep in range(repeat):
          col = 0
          for bi, b in enumerate(order):
            ctx_b = int(ctx_lens[b])
            nb, nk, nv = nbs[b], nks[b], nvs[b]
            nhg = nk // 64
            nvb = _round_up(nv, 128) // 128  # V out blocks actually filled
            rem = ctx_b % BLOCK

            ktile = kpool.tile([128, 16 * nk], mybir.dt.float8e3, tag="kt")
            # gather AP (shape-check only); physical layout is [d, j, page, c]
            kap_g = ktile[:].rearrange("p (t n) -> p t n", t=16)
            # contiguous view for the QK stationary: [d, j, (page c)]
            kap = ktile[:].rearrange("p (j m) -> p j m", j=8)
            vtile = vpool.tile([128, nvb * 8 * D], vdtype, tag="vt")
            vap = vtile[:].rearrange("p (g n) -> p g n", g=nvb)
            if qmap.startswith("m"):
                kqn = (2 * bi) % n_queues
                vqn = (2 * bi + 1) % n_queues
            elif qmap:
                kqn = bi % nkq
                vqn = nkq + bi % nvq
            else:
                kqn, vqn = 0, vq
            if isinstance(sp, str) and len(sp) == 2:
                sp_k, sp_v = int(sp[0]), int(sp[1])
            else:
                sp_k = sp_v = int(sp)
            if mode != "compute":
                if mode != "dmav":
                    nc.gpsimd.dma_gather(
                        out_ap=kap_g,
                        in_ap=k_src[:],
                        idxs_ap=idx_sb[:, col:col + nk // 16],
                        num_idxs=nk,
                        num_idxs_reg=nb,
                        elem_size=BLOCK * D,
                        transpose=True,
                        queue_num=kqn,
                        single_packet=bool(sp_k),
                    )
                if mode != "dmak":
                    nc.gpsimd.dma_gather(
                        out_ap=vap,
                        in_ap=v_src[:],
                        idxs_ap=idx_sb[:, col + nk // 16:col + nk // 16
                                       + nv // 16],
                        num_idxs=nv,
                        num_idxs_reg=2 * nb,
                        elem_size=8 * D,
                        transpose=False,
                        queue_num=vqn,
                        single_packet=bool(sp_v),
                    )
            else:
                # compute mode: every logical tile needs a write for Tile
                # validation; a sliver is enough (timing only, not numerics)
                nc.vector.memset(ktile[:, 0:16], 0.0)
                nc.vector.memset(vtile[:, 0:16], 0.0)
            col += nk // 16 + nv // 16
            if mode.startswith("dma"):
                # minimal consumer so the gathers aren't dead: copy a sliver
                sliver = epool.tile([128, 4], mybir.dt.float32, tag="slv")
                if mode != "dmav":
                    nc.vector.tensor_copy(out=sliver[:], in_=kap[:, 0, 0:4])
                if mode != "dmak":
                    nc.vector.tensor_copy(out=sliver[:], in_=vap[:, 0, 0:4])
                if bi == B - 1:
                    fin0 = epool.tile([G, D], mybir.dt.float32, tag="fin")
                    nc.vector.memset(fin0[:], 0.0)
                    for bb in range(B):
                        nc.sync.dma_start(out=out_d[bb], in_=fin0[:])
                continue

            out_ps = opsum.tile([128, G], mybir.dt.float32, tag="ops")
            den_ps = dpsum.tile([8 * G, 1], mybir.dt.float32, tag="dps")

            hgs = [hg for hg in range(nhg)
                   if any(_jt2(ctx_b, hg, j) > 0 for j in range(8))]
            last_hg = hgs[-1]
            n_pv = sum(1 for hg in hgs for j in range(8)
                       if _jt2(ctx_b, hg, j) > 0)
            pv_i = 0
            first_pv = True
            for hgi, hg in enumerate(hgs):
                scores_ps = spsum.tile([128, 8 * G], mybir.dt.float32, tag="sps")
                for j in range(8):
                    if _jt2(ctx_b, hg, j) == 0:
                        continue
                    nc.tensor.matmul(
                        scores_ps[:, 4 * j:4 * j + 4],
                        lhsT=kap[:, j, 128 * hg:128 * hg + 128],
                        rhs=qT_b[:, 4 * b:4 * b + 4],
                        start=True, stop=True,
                    )
                # is any (row, col) of this half-group's p tile garbage?
                partial = (hg == last_hg) and (2 * nb - 128 * hg < 128
                                               or rem > 0)
                ptile = ppool.tile([128, 8 * G], mybir.dt.bfloat16, tag="pt")
                if not partial:
                    nc.scalar.activation(ptile[:], scores_ps[:],
                                         mybir.ActivationFunctionType.Exp)
                else:
                    # exp into a temp, then keep only in-context entries so
                    # garbage (possibly NaN/inf) never reaches den/PV.
                    ptmp = ppool.tile([128, 8 * G], mybir.dt.bfloat16, tag="ptmp")
                    nc.scalar.activation(ptmp[:], scores_ps[:],
                                         mybir.ActivationFunctionType.Exp)
                    msk = epool.tile([128, 8 * G], mybir.dt.int8, tag="msk")
                    nc.sync.dma_start(out=msk[:], in_=mask_d[b])
                    nc.vector.memset(ptile[:], 0.0)
                    nc.vector.copy_predicated(ptile[:], msk[:], ptmp[:])
                # denominator contribution of this half-group
                nc.tensor.matmul(
                    den_ps[:],
                    lhsT=ptile[:],
                    rhs=ones_b[:],
                    start=(hgi == 0), stop=(hg == last_hg),
                )
                # PV accumulation: V stationary (128-col weight -> FWL), p
                # streams as the 4-col moving operand.
                for j in range(8):
                    jt = _jt2(ctx_b, hg, j)
                    if jt == 0:
                        continue
                    pv_i += 1
                    nc.tensor.matmul(
                        out_ps[:],
                        lhsT=vap[0:jt, hg, D * j:D * j + D],
                        rhs=ptile[0:jt, 4 * j:4 * j + 4],
                        start=first_pv, stop=(pv_i == n_pv),
                    )
                    first_pv = False

            # epilogue: out_ps [128,4] -> transpose -> scale by 1/den -> stage
            o_sb = epool.tile([128, G], mybir.dt.float32, tag="osb")
            nc.vector.tensor_copy(out=o_sb[:], in_=out_ps[:])
            oT_ps = tpsum.tile([G, 128], mybir.dt.float32, tag="otp")
            nc.tensor.transpose(oT_ps[:], o_sb[:], ident[:])

            den_sb = epool.tile([8 * G, 1], mybir.dt.float32, tag="dsb")
            nc.vector.tensor_copy(out=den_sb[:], in_=den_ps[:])
            den4_ps = npsum.tile([G, 1], mybir.dt.float32, tag="d4p")
            nc.tensor.matmul(den4_ps[:], lhsT=sel_sb[:], rhs=den_sb[:],
                             start=True, stop=True)
            den4_sb = epool.tile([G, 1], mybir.dt.float32, tag="d4s")
            nc.vector.tensor_copy(out=den4_sb[:], in_=den4_ps[:])
            rcp = epool.tile([G, 1], mybir.dt.float32, tag="rcp")
            nc.vector.reciprocal(rcp[:], den4_sb[:])

            fin = epool.tile([G, D], mybir.dt.float32, tag="fin")
            nc.vector.tensor_tensor(
                out=fin[:],
                in0=oT_ps[:],
                in1=rcp[:].to_broadcast([G, D]),
                op=mybir.AluOpType.mult,
            )
            nc.sync.dma_start(out=out_d[b], in_=fin[:])

    nc.compile()
    return nc


def _prep_host(q, k_cache, v_cache, block_tables, context_lens, vdt="bf16"):
    """Shard + reformat inputs for the 8 cores. Returns in_maps list."""
    VDT = F8E3 if vdt == "fp8" else BF16
    ctx_lens = np.asarray(context_lens, dtype=np.int64)
    bt = np.asarray(block_tables, dtype=np.int64)
    nbs = [max(1, -(-int(c) // BLOCK)) for c in ctx_lens]
    nks = [_round_up(nb, 128) for nb in nbs]
    nvs = [_round_up(2 * nb, 16) for nb in nbs]
    idx_cols = sum(nk // 16 + nv // 16 for nk, nv in zip(nks, nvs))

    # idx columns are packed in the same largest-first order the graph
    # builder iterates sequences in (see _build_graph): K block then V block.
    order = list(np.argsort(-np.asarray(nbs), kind="stable"))
    idx16 = np.full((16, idx_cols), -1, dtype=np.int16)
    col = 0
    for b in order:
        nb, nk, nv = nbs[b], nks[b], nvs[b]
        # ascending physical order for the full pages (HBM locality);
        # attention is token-permutation invariant. The partial last page
        # must stay last so the tail masking logic is unchanged.
        fb = int(ctx_lens[b]) // BLOCK
        pages = np.sort(bt[b, :fb])
        if nb > fb:
            pages = np.concatenate([pages, bt[b, fb:nb]])
        ids = np.full(nk, -1, dtype=np.int16)
        ids[:nb] = pages.astype(np.int16)
        idx16[:, col:col + nk // 16] = ids.reshape(nk // 16, 16).T
        col += nk // 16
        iv = np.full(nv, -1, dtype=np.int16)
        iv[0:2 * nb:2] = (2 * pages).astype(np.int16)
        iv[1:2 * nb:2] = (2 * pages + 1).astype(np.int16)
        idx16[:, col:col + nv // 16] = iv.reshape(nv // 16, 16).T
        col += nv // 16
    idx_all = np.tile(idx16, (8, 1))  # replicate across the 8 Q7 cores

    sel = np.zeros((8 * G, G), dtype=np.float32)
    for i in range(8 * G):
        sel[i, i % G] = 1.0

    # validity mask of the LAST half-group of each sequence:
    # mask[b, r, 4j+g] = 1 iff token (page 64*hg + r//2, slot 2j + r%2) < ctx
    mask = np.zeros((B, 128, 8 * G), dtype=np.int8)
    rv = np.arange(128)
    jv = np.arange(8)
    for b in range(B):
        ctx_b = int(ctx_lens[b])
        hg = (nbs[b] - 1) // 64
        pos = (BLOCK * (64 * hg + rv[:, None] // 2)
               + 2 * jv[None, :] + (rv[:, None] % 2))  # [128, 8]
        m = (pos < ctx_b).astype(np.int8)
        mask[b] = np.repeat(m, G, axis=1)

    q = np.asarray(q, dtype=np.float32)
    kc = np.asarray(k_cache, dtype=np.float32)
    vc = np.asarray(v_cache, dtype=np.float32)

    in_maps = []
    for c in range(KVH):
        ks = np.ascontiguousarray(kc[:, :, c, :])  # [NBLK, 16, 128] f32
        # page layout (t, d) -> (t//2, d, t%2) so the 16-bit-granularity
        # transposed gather lands K^T as [d, j, page, t%2]
        k_shard = np.ascontiguousarray(
            ks.reshape(NBLK, 8, 2, 128).transpose(0, 1, 3, 2)
        ).astype(F8E3).reshape(NBLK, BLOCK * D)
        vs = np.ascontiguousarray(vc[:, :, c, :]).astype(VDT)  # [NBLK,16,128]
        v_shard = np.empty((2 * NBLK, 8 * D), dtype=VDT)
        v_shard[0::2] = vs[:, 0::2, :].reshape(NBLK, 8 * D)
        v_shard[1::2] = vs[:, 1::2, :].reshape(NBLK, 8 * D)
        qs = np.ascontiguousarray(q[:, G * c:G * c + G, :] * SCALE)  # [32,4,128]
        qT = np.ascontiguousarray(qs.reshape(B * G, D).T.astype(np.float32))
        in_maps.append({
            "k_src": k_shard,
            "v_src": v_shard,
            "qT": qT,
            "idx": idx_all,
            "sel": sel,
            "mask": mask,
        })
    return in_maps


def _get_graph(context_lens, repeat=1, bufs=(3, 3, 3, 2, 2, 2), mode="full",
               vq=1, vdt="bf16", sp=1, qmap=""):
    key = (bytes(np.asarray(context_lens, dtype=np.int32)), repeat, bufs, mode,
           vq, vdt, sp, qmap)
    if key not in _GRAPH_CACHE:
        _GRAPH_CACHE[key] = _build_graph(
            np.asarray(context_lens, dtype=np.int64), repeat=repeat, bufs=bufs,
            mode=mode, vq=vq, vdt=vdt, sp=sp, qmap=qmap)
    return _GRAPH_CACHE[key]


# best-known configuration (updated as measurements come in)
BEST = dict(qmap="k1v3", sp="10", vdt="bf16")


def kernel_run(q, k_cache, v_cache, block_tables, context_lens, trace=False):
    """Run on the 8 NeuronCores; returns (out, BassKernelResults)."""
    import time
    from concourse.bass_utils import run_bass_kernel_spmd

    nc = _get_graph(context_lens, **BEST)
    in_maps = _prep_host(q, k_cache, v_cache, block_tables, context_lens,
                         vdt=BEST["vdt"])
    last_exc = None
    for attempt in range(3):
        try:
            res = run_bass_kernel_spmd(nc, in_maps, core_ids=list(range(8)),
                                       trace=trace)
            break
        except Exception as e:  # transient device wedge (e.g. NRT_EXEC_UNIT_
            last_exc = e        # UNRECOVERABLE) — back off and retry
            time.sleep(5 * (attempt + 1))
    else:
        raise last_exc
    outs = [np.asarray(r["out"], dtype=np.float32) for r in res.results]
    out = np.concatenate(outs, axis=1).reshape(B, H, D)
    return out, res


def kernel(q, k_cache, v_cache, block_tables, context_lens):
    out, _ = kernel_run(q, k_cache, v_cache, block_tables, context_lens,
                        trace=False)
    return out



# revision 18
# speedup vs baseline: 1.7433x; 1.2667x over previous
"""Paged-attention decode kernel for 8 TRN2 NeuronCores (Bass/Tile).

Problem: nn_Attention_15229954031958 (sparse_attention, memory-bound).
  q [32, 32, 128] f32, k/v_cache [8192, 16, 8, 128] f32,
  block_tables [32, 256] i32, context_lens [32] i32 -> out [32, 32, 128] f32.

Sharding: tensor-parallel over KV heads. Core c holds the head-c slice of
both caches plus q heads 4c..4c+3; no cross-core communication. Every
core runs the same graph (trip counts baked from context_lens, identical
on all cores), so SPMD is trivially satisfied.

Layout ("ilv" scheme). K is stored fp8-e3m4, one 2KB row per page, host
pre-swizzled as (t, d) -> (t//2, d, t%2). The transposed dma_gather moves
16-bit units through the xbar, so the gathered K tile lands as
[d=128, j=8, page, c=2] with token t = 2j + c; a flat 128-column slice
over (page, c) is CONTIGUOUS, giving the QK matmul an FWL-eligible
stationary operand. Score rows are therefore (page, parity)-interleaved
pairs over 64-page half-groups. V is stored bf16 as split half-pages
(row 2i = even tokens of page i, row 2i+1 = odd tokens); its natural
gather with interleaved indices produces V rows in exactly the same
(page, parity) order, so the PV contraction lines up row-for-row.

Gather plumbing (this session's optimization): the kernel is bound by the
SWDGE gather path, not compute (compute-only mode measures ~0 marginal).
Three levers applied, worth ~147.9us -> ~112us contended / ~49-59us when
the sibling NeuronCores' HBM stacks are idle:
  - V gathers round-robin over SWDGE queues 1-3, K on queue 0 (qmap
    "k1v3"). Each queue's descriptor generation runs on its own Q7 core
    pair (queue q -> cores 2q, 2q+1), so splitting V's 2nb descriptors
    per seq over 3 queues parallelizes the gen that used to serialize on
    one core pair (V-gather-only: 168us -> 49us).
  - V's idx stream is exact-size (2nb rounded to 16) instead of padded
    to 2*ceil128(nb); the gather ucode pops the static num_idxs count of
    idxs before trimming trailing -1s, so padding costs Q7 time. K must
    stay %128 (transpose-gather constraint); its trailing -1s are
    trimmed at runtime and cost no HBM traffic.
  - Full pages are sorted ascending (per seq, partial page kept last so
    the tail masks still work): ascending 2KB reads give the HBM row
    buffers a break; attention is token-permutation invariant.
  - single_packet: K=1 (concatenates the 256B xbar spray descriptors,
    amortizing per-packet overhead), V=0 (2KB descs are near line rate
    unpacked and round-robin fairly across queues). sp="01" measured
    pathological (204us) - do not flip.
Measurement warning: device throughput swings ~2.5x between sessions
(sibling-NC HBM contention); the same config measured 49-136us. Compare
configs only within one anchored process, and prefer the contended-
regime winner.

Per sequence (nb = ceil(ctx/16) pages, nhg = ceil2(nb)/64 half-groups):
  - per half-group hg and slot-pair j: scores[128, 4] =
      ktile[:, j, 128hg:128hg+128]^T @ qT[:, 4b:4b+4]   (PSUM [128, 32])
  - one batched exp per half-group on ScalarE -> bf16 p
  - PV: out[128, 4] += vtile[0:jt2, hg, 128j:+128]^T @ p[0:jt2, 4j:4j+4]
    with jt2 the static count of leading valid (page, parity) rows;
    V stationary (128-col weight -> FWL), p streams 4 cols.
  - den: p^T @ ones -> [32, 1] PSUM accumulated over half-groups,
    collapsed to [4, 1] with a selector matmul; epilogue transposes the
    PV accumulator on the PE and scales by 1/den.
"""

import os
import numpy as np
import ml_dtypes

BLOCK = 16
D = 128
B = 32
H = 32
KVH = 8
G = 4
NBLK = 8192
MAXB = 256
SCALE = 0.08838834764831845
BF16 = ml_dtypes.bfloat16
F8E3 = ml_dtypes.float8_e3m4

_GRAPH_CACHE = {}


def _round_up(x, m):
    return (x + m - 1) // m * m


def _jt2(ctx, hg, j):
    """Leading valid (page, parity) rows of half-group hg, slot-pair j."""
    fb, rem = ctx // BLOCK, ctx % BLOCK
    n0 = min(max(fb + (1 if 2 * j < rem else 0) - 64 * hg, 0), 64)
    n1 = min(max(fb + (1 if 2 * j + 1 < rem else 0) - 64 * hg, 0), 64)
    return n0 + n1


def _build_graph(ctx_lens, repeat=1, bufs=(3, 3, 3, 2, 2, 2), mode="full",
                 vq=1, vdt="bf16", sp=1, qmap=""):
    """Build + compile the SPMD graph for the given context lengths.

    repeat > 1 duplicates the whole body (for timing: slope difference
    between repeat=R and repeat=1 isolates pure HW time).
    mode: "full" | "dma"/"dmak"/"dmav" (gathers only) | "compute".
    vq: SWDGE queue for the V gather (K is always queue 0).
    vdt: "bf16" | "fp8" (e3m4) for the V cache.
    qmap: "kAvB" -> K gathers round-robin on queues [0, A), V gathers on
    queues [A, A+B). Overrides vq. Empty = legacy (K on 0, V on vq)."""
    import concourse.bass as bass
    import concourse.tile as tile
    from concourse import bacc, mybir
    from concourse.masks import make_identity
    from contextlib import ExitStack

    kb, vb, pb, eb, sb, ob = bufs
    nbs = [max(1, -(-int(c) // BLOCK)) for c in ctx_lens]
    nks = [_round_up(nb, 128) for nb in nbs]
    # V idx stream is exact-size (2nb rounded to 16), not nk-padded
    nvs = [_round_up(2 * nb, 16) for nb in nbs]
    # idx columns per seq: nk/16 for K, nv/16 for V
    idx_cols = sum(nk // 16 + nv // 16 for nk, nv in zip(nks, nvs))
    # process sequences largest-first: best prefetch ramp at the start and a
    # minimal non-overlapped tail (last gather is the smallest sequence)
    order = list(np.argsort(-np.asarray(nbs), kind="stable"))

    if qmap.startswith("m"):
        nkq, nvq = 0, 0
        n_queues = int(qmap[1])
    elif qmap:
        nkq, nvq = int(qmap[1]), int(qmap[3])
        n_queues = nkq + nvq
    else:
        nkq, nvq = 0, 0
        n_queues = 2 if vq else 1
    nc = bacc.Bacc("TRN2", target_bir_lowering=False, debug=False,
                   num_swdge_queues=n_queues)

    vdtype = mybir.dt.float8e3 if vdt == "fp8" else mybir.dt.bfloat16
    k_src = nc.dram_tensor("k_src", [NBLK, BLOCK * D], mybir.dt.float8e3,
                           kind="ExternalInput").ap()
    v_src = nc.dram_tensor("v_src", [2 * NBLK, 8 * D], vdtype,
                           kind="ExternalInput").ap()
    qT_d = nc.dram_tensor("qT", [D, B * G], mybir.dt.float32,
                          kind="ExternalInput").ap()
    idx_d = nc.dram_tensor("idx", [128, idx_cols], mybir.dt.int16,
                           kind="ExternalInput").ap()
    sel_d = nc.dram_tensor("sel", [8 * G, G], mybir.dt.float32,
                           kind="ExternalInput").ap()
    mask_d = nc.dram_tensor("mask", [B, 128, 8 * G], mybir.dt.int8,
                            kind="ExternalInput").ap()
    out_d = nc.dram_tensor("out", [B, G, D], mybir.dt.float32,
                           kind="ExternalOutput").ap()

    with tile.TileContext(nc) as tc, ExitStack() as ctx:
        const = ctx.enter_context(tc.tile_pool(name="const", bufs=1))
        kpool = ctx.enter_context(tc.tile_pool(name="kpool", bufs=kb))
        vpool = ctx.enter_context(tc.tile_pool(name="vpool", bufs=vb))
        ppool = ctx.enter_context(tc.tile_pool(name="ppool", bufs=pb))
        epool = ctx.enter_context(tc.tile_pool(name="epool", bufs=eb))
        spsum = ctx.enter_context(tc.tile_pool(name="spsum", bufs=sb, space="PSUM"))
        opsum = ctx.enter_context(tc.tile_pool(name="opsum", bufs=ob, space="PSUM"))
        dpsum = ctx.enter_context(tc.tile_pool(name="dpsum", bufs=1, space="PSUM"))
        tpsum = ctx.enter_context(tc.tile_pool(name="tpsum", bufs=1, space="PSUM"))
        npsum = ctx.enter_context(tc.tile_pool(name="npsum", bufs=1, space="PSUM"))

        qT_sb = const.tile([128, B * G], mybir.dt.float32)
        nc.sync.dma_start(out=qT_sb[:], in_=qT_d[:])
        qT_b = const.tile([128, B * G], mybir.dt.bfloat16)
        nc.vector.tensor_copy(out=qT_b[:], in_=qT_sb[:])

        ident = const.tile([128, 128], mybir.dt.float32)
        make_identity(nc, ident[:])

        ones_b = const.tile([128, 1], mybir.dt.bfloat16)
        nc.vector.memset(ones_b[:], 1.0)

        sel_sb = const.tile([8 * G, G], mybir.dt.float32)
        nc.sync.dma_start(out=sel_sb[:], in_=sel_d[:])

        idx_sb = const.tile([128, idx_cols], mybir.dt.int16)
        nc.sync.dma_start(out=idx_sb[:], in_=idx_d[:])

        for _rep in range(repeat):
          col = 0
          for bi, b in enumerate(order):
            ctx_b = int(ctx_lens[b])
            nb, nk, nv = nbs[b], nks[b], nvs[b]
            nhg = nk // 64
            nvb = _round_up(nv, 128) // 128  # V out blocks actually filled
            rem = ctx_b % BLOCK

            ktile = kpool.tile([128, 16 * nk], mybir.dt.float8e3, tag="kt")
            # gather AP (shape-check only); physical layout is [d, j, page, c]
            kap_g = ktile[:].rearrange("p (t n) -> p t n", t=16)
            # contiguous view for the QK stationary: [d, j, (page c)]
            kap = ktile[:].rearrange("p (j m) -> p j m", j=8)
            vtile = vpool.tile([128, nvb * 8 * D], vdtype, tag="vt")
            vap = vtile[:].rearrange("p (g n) -> p g n", g=nvb)
            if qmap.startswith("m"):
                kqn = (2 * bi) % n_queues
                vqn = (2 * bi + 1) % n_queues
            elif qmap:
                kqn = bi % nkq
                vqn = nkq + bi % nvq
            else:
                kqn, vqn = 0, vq
            if isinstance(sp, str) and len(sp) == 2:
                sp_k, sp_v = int(sp[0]), int(sp[1])
            else:
                sp_k = sp_v = int(sp)
            if mode != "compute":
                if mode != "dmav":
                    nc.gpsimd.dma_gather(
                        out_ap=kap_g,
                        in_ap=k_src[:],
                        idxs_ap=idx_sb[:, col:col + nk // 16],
                        num_idxs=nk,
                        num_idxs_reg=nb,
                        elem_size=BLOCK * D,
                        transpose=True,
                        queue_num=kqn,
                        single_packet=bool(sp_k),
                    )
                if mode != "dmak":
                    nc.gpsimd.dma_gather(
                        out_ap=vap,
                        in_ap=v_src[:],
                        idxs_ap=idx_sb[:, col + nk // 16:col + nk // 16
                                       + nv // 16],
                        num_idxs=nv,
                        num_idxs_reg=2 * nb,
                        elem_size=8 * D,
                        transpose=False,
                        queue_num=vqn,
                        single_packet=bool(sp_v),
                    )
            else:
                # compute mode: every logical tile needs a write for Tile
                # validation; a sliver is enough (timing only, not numerics)
                nc.vector.memset(ktile[:, 0:16], 0.0)
                nc.vector.memset(vtile[:, 0:16], 0.0)
            col += nk // 16 + nv // 16
            if mode.startswith("dma"):
                # minimal consumer so the gathers aren't dead: copy a sliver
                sliver = epool.tile([128, 4], mybir.dt.float32, tag="slv")
                if mode != "dmav":
                    nc.vector.tensor_copy(out=sliver[:], in_=kap[:, 0, 0:4])
                if mode != "dmak":
                    nc.vector.tensor_copy(out=sliver[:], in_=vap[:, 0, 0:4])
                if bi == B - 1:
                    fin0 = epool.tile([G, D], mybir.dt.float32, tag="fin")
                    nc.vector.memset(fin0[:], 0.0)
                    for bb in range(B):
                        nc.sync.dma_start(out=out_d[bb], in_=fin0[:])
                continue

            out_ps = opsum.tile([128, G], mybir.dt.float32, tag="ops")
            den_ps = dpsum.tile([8 * G, 1], mybir.dt.float32, tag="dps")

            hgs = [hg for hg in range(nhg)
                   if any(_jt2(ctx_b, hg, j) > 0 for j in range(8))]
            last_hg = hgs[-1]
            n_pv = sum(1 for hg in hgs for j in range(8)
                       if _jt2(ctx_b, hg, j) > 0)
            pv_i = 0
            first_pv = True
            for hgi, hg in enumerate(hgs):
                scores_ps = spsum.tile([128, 8 * G], mybir.dt.float32, tag="sps")
                for j in range(8):
                    if _jt2(ctx_b, hg, j) == 0:
                        continue
                    nc.tensor.matmul(
                        scores_ps[:, 4 * j:4 * j + 4],
                        lhsT=kap[:, j, 128 * hg:128 * hg + 128],
                        rhs=qT_b[:, 4 * b:4 * b + 4],
                        start=True, stop=True,
                    )
                # is any (row, col) of this half-group's p tile garbage?
                partial = (hg == last_hg) and (2 * nb - 128 * hg < 128
                                               or rem > 0)
                ptile = ppool.tile([128, 8 * G], mybir.dt.bfloat16, tag="pt")
                if not partial:
                    nc.scalar.activation(ptile[:], scores_ps[:],
                                         mybir.ActivationFunctionType.Exp)
                else:
                    # exp into a temp, then keep only in-context entries so
                    # garbage (possibly NaN/inf) never reaches den/PV.
                    ptmp = ppool.tile([128, 8 * G], mybir.dt.bfloat16, tag="ptmp")
                    nc.scalar.activation(ptmp[:], scores_ps[:],
                                         mybir.ActivationFunctionType.Exp)
                    msk = epool.tile([128, 8 * G], mybir.dt.int8, tag="msk")
                    nc.sync.dma_start(out=msk[:], in_=mask_d[b])
                    nc.vector.memset(ptile[:], 0.0)
                    nc.vector.copy_predicated(ptile[:], msk[:], ptmp[:])
                # denominator contribution of this half-group
                nc.tensor.matmul(
                    den_ps[:],
                    lhsT=ptile[:],
                    rhs=ones_b[:],
                    start=(hgi == 0), stop=(hg == last_hg),
                )
                # PV accumulation: V stationary (128-col weight -> FWL), p
                # streams as the 4-col moving operand.
                for j in range(8):
                    jt = _jt2(ctx_b, hg, j)
                    if jt == 0:
                        continue
                    pv_i += 1
                    nc.tensor.matmul(
                        out_ps[:],
                        lhsT=vap[0:jt, hg, D * j:D * j + D],
                        rhs=ptile[0:jt, 4 * j:4 * j + 4],
                        start=first_pv, stop=(pv_i == n_pv),
                    )
                    first_pv = False

            # epilogue: out_ps [128,4] -> transpose -> scale by 1/den -> stage
            o_sb = epool.tile([128, G], mybir.dt.float32, tag="osb")
            nc.vector.tensor_copy(out=o_sb[:], in_=out_ps[:])
            oT_ps = tpsum.tile([G, 128], mybir.dt.float32, tag="otp")
            nc.tensor.transpose(oT_ps[:], o_sb[:], ident[:])

            den_sb = epool.tile([8 * G, 1], mybir.dt.float32, tag="dsb")
            nc.vector.tensor_copy(out=den_sb[:], in_=den_ps[:])
            den4_ps = npsum.tile([G, 1], mybir.dt.float32, tag="d4p")
            nc.tensor.matmul(den4_ps[:], lhsT=sel_sb[:], rhs=den_sb[:],
                             start=True, stop=True)
            den4_sb = epool.tile([G, 1], mybir.dt.float32, tag="d4s")
            nc.vector.tensor_copy(out=den4_sb[:], in_=den4_ps[:])
            rcp = epool.tile([G, 1], mybir.dt.float32, tag="rcp")
            nc.vector.reciprocal(rcp[:], den4_sb[:])

            fin = epool.tile([G, D], mybir.dt.float32, tag="fin")
            nc.vector.tensor_tensor(
                out=fin[:],
                in0=oT_ps[:],
                in1=rcp[:].to_broadcast([G, D]),
                op=mybir.AluOpType.mult,
            )
            nc.sync.dma_start(out=out_d[b], in_=fin[:])

    nc.compile()
    return nc


def _prep_host(q, k_cache, v_cache, block_tables, context_lens, vdt="bf16"):
    """Shard + reformat inputs for the 8 cores. Returns in_maps list."""
    VDT = F8E3 if vdt == "fp8" else BF16
    ctx_lens = np.asarray(context_lens, dtype=np.int64)
    bt = np.asarray(block_tables, dtype=np.int64)
    nbs = [max(1, -(-int(c) // BLOCK)) for c in ctx_lens]
    nks = [_round_up(nb, 128) for nb in nbs]
    nvs = [_round_up(2 * nb, 16) for nb in nbs]
    idx_cols = sum(nk // 16 + nv // 16 for nk, nv in zip(nks, nvs))

    # idx columns are packed in the same largest-first order the graph
    # builder iterates sequences in (see _build_graph): K block then V block.
    order = list(np.argsort(-np.asarray(nbs), kind="stable"))
    idx16 = np.full((16, idx_cols), -1, dtype=np.int16)
    col = 0
    for b in order:
        nb, nk, nv = nbs[b], nks[b], nvs[b]
        # ascending physical order for the full pages (HBM locality);
        # attention is token-permutation invariant. The partial last page
        # must stay last so the tail masking logic is unchanged.
        fb = int(ctx_lens[b]) // BLOCK
        pages = np.sort(bt[b, :fb])
        if nb > fb:
            pages = np.concatenate([pages, bt[b, fb:nb]])
        ids = np.full(nk, -1, dtype=np.int16)
        ids[:nb] = pages.astype(np.int16)
        idx16[:, col:col + nk // 16] = ids.reshape(nk // 16, 16).T
        col += nk // 16
        iv = np.full(nv, -1, dtype=np.int16)
        iv[0:2 * nb:2] = (2 * pages).astype(np.int16)
        iv[1:2 * nb:2] = (2 * pages + 1).astype(np.int16)
        idx16[:, col:col + nv // 16] = iv.reshape(nv // 16, 16).T
        col += nv // 16
    idx_all = np.tile(idx16, (8, 1))  # replicate across the 8 Q7 cores

    sel = np.zeros((8 * G, G), dtype=np.float32)
    for i in range(8 * G):
        sel[i, i % G] = 1.0

    # validity mask of the LAST half-group of each sequence:
    # mask[b, r, 4j+g] = 1 iff token (page 64*hg + r//2, slot 2j + r%2) < ctx
    mask = np.zeros((B, 128, 8 * G), dtype=np.int8)
    rv = np.arange(128)
    jv = np.arange(8)
    for b in range(B):
        ctx_b = int(ctx_lens[b])
        hg = (nbs[b] - 1) // 64
        pos = (BLOCK * (64 * hg + rv[:, None] // 2)
               + 2 * jv[None, :] + (rv[:, None] % 2))  # [128, 8]
        m = (pos < ctx_b).astype(np.int8)
        mask[b] = np.repeat(m, G, axis=1)

    q = np.asarray(q, dtype=np.float32)
    kc = np.asarray(k_cache, dtype=np.float32)
    vc = np.asarray(v_cache, dtype=np.float32)

    in_maps = []
    for c in range(KVH):
        ks = np.ascontiguousarray(kc[:, :, c, :])  # [NBLK, 16, 128] f32
        # page layout (t, d) -> (t//2, d, t%2) so the 16-bit-granularity
        # transposed gather lands K^T as [d, j, page, t%2]
        k_shard = np.ascontiguousarray(
            ks.reshape(NBLK, 8, 2, 128).transpose(0, 1, 3, 2)
        ).astype(F8E3).reshape(NBLK, BLOCK * D)
        vs = np.ascontiguousarray(vc[:, :, c, :]).astype(VDT)  # [NBLK,16,128]
        v_shard = np.empty((2 * NBLK, 8 * D), dtype=VDT)
        v_shard[0::2] = vs[:, 0::2, :].reshape(NBLK, 8 * D)
        v_shard[1::2] = vs[:, 1::2, :].reshape(NBLK, 8 * D)
        qs = np.ascontiguousarray(q[:, G * c:G * c + G, :] * SCALE)  # [32,4,128]
        qT = np.ascontiguousarray(qs.reshape(B * G, D).T.astype(np.float32))
        in_maps.append({
            "k_src": k_shard,
            "v_src": v_shard,
            "qT": qT,
            "idx": idx_all,
            "sel": sel,
            "mask": mask,
        })
    return in_maps


def _get_graph(context_lens, repeat=1, bufs=(3, 3, 3, 2, 2, 2), mode="full",
               vq=1, vdt="bf16", sp=1, qmap=""):
    key = (bytes(np.asarray(context_lens, dtype=np.int32)), repeat, bufs, mode,
           vq, vdt, sp, qmap)
    if key not in _GRAPH_CACHE:
        _GRAPH_CACHE[key] = _build_graph(
            np.asarray(context_lens, dtype=np.int64), repeat=repeat, bufs=bufs,
            mode=mode, vq=vq, vdt=vdt, sp=sp, qmap=qmap)
    return _GRAPH_CACHE[key]


# best-known configuration (updated as measurements come in)
BEST = dict(qmap="k1v3", sp="10", vdt="bf16")


def kernel_run(q, k_cache, v_cache, block_tables, context_lens, trace=False):
    """Run on the 8 NeuronCores; returns (out, BassKernelResults)."""
    import time
    from concourse.bass_utils import run_bass_kernel_spmd

    nc = _get_graph(context_lens, **BEST)
    in_maps = _prep_host(q, k_cache, v_cache, block_tables, context_lens,
                         vdt=BEST["vdt"])
    last_exc = None
    for attempt in range(3):
        try:
            res = run_bass_kernel_spmd(nc, in_maps, core_ids=list(range(8)),
                                       trace=trace)
            break
        except Exception as e:  # transient device wedge (e.g. NRT_EXEC_UNIT_
            last_exc = e        # UNRECOVERABLE) — back off and retry
            time.sleep(5 * (attempt + 1))
    else:
        raise last_exc
    outs = [np.asarray(r["out"], dtype=np.float32) for r in res.results]
    out = np.concatenate(outs, axis=1).reshape(B, H, D)
    return out, res


def kernel(q, k_cache, v_cache, block_tables, context_lens):
    out, _ = kernel_run(q, k_cache, v_cache, block_tables, context_lens,
                        trace=False)
    return out

